# revision 75
# baseline (speedup 1.0000x reference)
"""Trainium2 Bass kernel: AttentionBlock (GroupNorm + self/cross QKV attention + proj + residual).

Data-parallel over batch: B=8, one batch element per NeuronCore (8 cores), no collectives.

Design (all numbers per core; C=768, T=1024, S=256, 12 heads x 64 ch):
  - Every matmul family runs fp8e4 with DoubleRow perf mode (2 k-tiles per
    instruction, 0.5 cycles/row): qkv/enc/proj contract over C=768 as
    3x(2x128) with host-prepped [128, ktp, 2, m] interleaved weights and
    [128, 2, n] interleaved activations; attention scores contract over
    ch=64 as 2x32, with q/k produced in fp8 (32x scaled) and rearranged
    into [32, head, ktile, t] by partition-offset block DMAs on idle
    queues. The 32x32 score scale folds into the exp.
  - exp() is split across ScalarE (true Exp) and DVE (Schraudolph int16
    bit-trick emitting bf16 bits directly, ~3.3% max mult err); GPSIMD
    cannot read PSUM so it takes only SBUF-side work (Z broadcast,
    normalize multiplies, vt copies are ACT/DVE).
  - Pipeline: each head pair is processed in two t-halves (120 steps of
    one s-tile x two heads), so only 2 PV accumulators [65,512] are live
    at once. PSUM: scores 4x[128,512] (two steps of drain slack) +
    PV 2 + insertions (q/k/vt production) 1x[128,1024] = 8 banks.
    PV lags scores by 2 steps; q/k/vt production interleaves through its
    own PSUM slot so it never churns the scores rotation.
  - PV uses vt (with a ones column -> softmax denominator Z in row 64)
    as stationary over E; normalization stages pa into SBUF (frees banks),
    takes 1/Z on DVE, broadcasts on GPSIMD, multiplies into fp8 a_dr.
  - proj consumes fp8 a_dr via DoubleRow; residual+store overlap the tail
    with two pre-opened partial chains and both HWDGE queues.
"""

import os
import numpy as np
import ml_dtypes
from contextlib import ExitStack

import concourse.tile as tile
from concourse import bacc, mybir
from concourse.bass_utils import run_bass_kernel_spmd

F32 = mybir.dt.float32
BF16 = mybir.dt.bfloat16
FP8 = mybir.dt.float8e4
I16 = mybir.dt.int16
NPBF = ml_dtypes.bfloat16
NPF8 = ml_dtypes.float8_e4m3

B, C, HH, WW = 8, 768, 32, 32
T = HH * WW          # 1024
S = 256
EC = 768
NH, CH = 12, 64      # heads, head channels
NG = 32              # groupnorm groups
EPS = 1e-5
NP = C // 128        # 6 channel-partition tiles
NPAIR = NH // 2      # 6 head pairs
ST = S + T           # 1280 attention keys
NS = ST // 128       # 10 s-tiles
NK = 3               # ktile-pairs for C=768 contraction (3 x (2x128))
SCALE = 1.0 / np.sqrt(np.sqrt(CH))

WS = 32.0            # fp8 weight scale for q/k/ek
WSV = 64.0           # fp8 weight scale for v/ev/proj
APS = 32.0           # fp8 prescale for attention output a

AOP = mybir.AluOpType
ACT = mybir.ActivationFunctionType
DR = mybir.MatmulPerfMode.DoubleRow

LN2 = float(np.log(2.0))
EXP_A = 128.0 / LN2                        # bf16-bits Schraudolph
EXP_B = 16256.0 - 128.0 * float(np.log2(1.0302))

# exp engine schedule, cycled over half-tiles (4 per step): a=ACT, d=DVE, g=GP
EXP_SCHED = os.environ.get("K_EXPSCHED", "aadadaadadaadadd")  # no g: gpsimd cannot read PSUM
VW = 65  # vt row width per head: 64 ch + ones column (softmax denominator)


def _emit(tc, ins, out_ap):
    nc = tc.nc
    ctx = tc._ctx

    # ---------------- pools ----------------
    const = ctx.enter_context(tc.tile_pool(name="const", bufs=1))
    xpool = ctx.enter_context(tc.tile_pool(name="x", bufs=1))
    attn = ctx.enter_context(tc.tile_pool(name="attn", bufs=1))
    spool = ctx.enter_context(tc.tile_pool(name="small", bufs=4))
    opool = ctx.enter_context(tc.tile_pool(name="o", bufs=1))
    epool = ctx.enter_context(tc.tile_pool(name="E", bufs=8))
    zpool = ctx.enter_context(tc.tile_pool(name="z", bufs=2))
    early = tc.alloc_tile_pool(name="early", bufs=1)
    sqpool = tc.alloc_tile_pool(name="sq", bufs=2)

    # ---------------- SBUF residents ----------------
    x_ct = [xpool.tile([128, T], F32, tag=f"x{i}", name=f"x_{i}") for i in range(NP)]
    xn_db = [xpool.tile([128, 2 * T], FP8, tag=f"xn{i}", name=f"xn_{i}")
             for i in range(NK)]
    q_f8 = [attn.tile([128, T], FP8, tag=f"q{j}", name=f"qf_{j}")
            for j in range(NPAIR)]
    k_f8 = [attn.tile([128, ST], FP8, tag=f"k{j}", name=f"kf_{j}")
            for j in range(NPAIR)]
    # DoubleRow-shuffled q/k: [32, head, ktile, t] (ch = 32*ktile + p)
    q_dr = [attn.tile([32, 2 * 2 * T], FP8, tag=f"qd{j}", name=f"qd_{j}")
            for j in range(NPAIR)]
    k_dr = [attn.tile([32, 2 * 2 * ST], FP8, tag=f"kd{j}", name=f"kd_{j}")
            for j in range(NPAIR)]
    # vt per s-tile: [p, h, ch+1] (ones column = softmax denominator)
    vt_st = [attn.tile([128, NH * VW], BF16, tag=f"vt{st}", name=f"vt_{st}")
             for st in range(NS)]
    a_dr = [attn.tile([128, 2 * T], FP8, tag=f"a{i}", name=f"a_{i}")
            for i in range(NK)]

    wq_sb = const.tile([128, NPAIR * NK * 2 * 128], FP8, tag="wq")
    wk_sb = const.tile([128, NPAIR * NK * 2 * 128], FP8, tag="wk")
    wv_sb = const.tile([128, NK * 2 * C], FP8, tag="wv")
    wek_sb = early.tile([128, NPAIR * NK * 2 * 128], FP8, tag="wek")
    wev_sb = early.tile([128, NK * 2 * C], FP8, tag="wev")
    wp_sb = const.tile([128, NK * 2 * C], FP8, tag="wp")
    enc_sb = early.tile([128, NK * 2 * S], FP8, tag="enc")

    bev_sb = early.tile([1, C], BF16, tag="bev")
    bqc_sb = const.tile([128, NPAIR], F32, tag="bqc")
    bkc_sb = const.tile([128, NPAIR], F32, tag="bkc")
    bekc_sb = const.tile([128, NPAIR], F32, tag="bekc")

    gnw_sb = const.tile([128, NP], F32, tag="gnw")
    gnb_sb = const.tile([128, NP], F32, tag="gnb")
    ind_sb = early.tile([128, NP * NG], F32, tag="ind")
    indT_sb = early.tile([32, C], F32, tag="indT")

    ones_r = early.tile([1, 128], BF16, tag="ones_r")
    zeros_c = const.tile([128, 1], F32, tag="zeros_c")

    s12_sb = const.tile([128, 2 * NP], F32, tag="s12")
    ab_sb = const.tile([128, 2 * NP], F32, tag="ab")

    # ---------------- input DMAs ----------------
    nc.vector.memset(ones_r[:], 1.0)
    nc.vector.memset(zeros_c[:], 0.0)
    warm_t = const.tile([1, 1], F32, tag="warm")
    nc.scalar.activation(warm_t[:], zeros_c[0:1, 0:1], ACT.Exp)

    # spread input DMAs over both HWDGE queues (SP + ACT) and gpsimd SWDGE
    # so descriptor generation doesn't serialize the startup; x tiles split
    # across both queues (groupnorm stats need all six as early as possible)
    for ct in range(NP):
        eng = nc.sync if ct < 4 else nc.gpsimd
        eng.dma_start(x_ct[ct][:], ins["x"][128 * ct: 128 * (ct + 1), :])
    for nm, dst in (("ind", ind_sb), ("indT", indT_sb), ("gnw", gnw_sb),
                    ("gnb", gnb_sb), ("bqc", bqc_sb), ("bkc", bkc_sb),
                    ("enc", enc_sb), ("wek", wek_sb)):
        nc.sync.dma_start(dst[:], ins[nm])
    for nm, dst in (("wev", wev_sb), ("bev", bev_sb), ("bekc", bekc_sb),
                    ("wq", wq_sb), ("wk", wk_sb), ("wv", wv_sb),
                    ("wp", wp_sb)):
        nc.gpsimd.dma_start(dst[:], ins[nm])

    # DR-layout views
    def w_pair_view(w, j):
        v = w[:].rearrange("p (j k i m) -> p j k i m", j=NPAIR, k=NK, i=2)
        return v[:, j]

    wv_v = wv_sb[:].rearrange("p (k i n) -> p k i n", k=NK, i=2)
    wev_v = wev_sb[:].rearrange("p (k i n) -> p k i n", k=NK, i=2)
    wp_v = wp_sb[:].rearrange("p (k i n) -> p k i n", k=NK, i=2)
    enc_v = enc_sb[:].rearrange("p (k i n) -> p k i n", k=NK, i=2)
    xn_v = [t[:].rearrange("p (i n) -> p i n", i=2) for t in xn_db]
    a_v = [t[:].rearrange("p (i n) -> p i n", i=2) for t in a_dr]
    vt3 = [t[:].rearrange("p (h c) -> p h c", h=NH) for t in vt_st]
    qd_v = [t[:].rearrange("p (hh ii n) -> p hh ii n", hh=2, ii=2) for t in q_dr]
    kd_v = [t[:].rearrange("p (hh ii n) -> p hh ii n", hh=2, ii=2) for t in k_dr]

    def emit_shuffle(j, which):
        # partition-offset block DMAs: rows 64*hh+32*ii -> partition-0 block
        src_t, dst_t, w, eng = (
            (q_f8[j], q_dr[j], T, nc.sync) if which == "q"
            else (k_f8[j], k_dr[j], ST, nc.gpsimd))
        for hh in range(2):
            for ii in range(2):
                r = 64 * hh + 32 * ii
                eng.dma_start(
                    dst_t[0:32, (2 * hh + ii) * w: (2 * hh + ii + 1) * w],
                    src_t[r: r + 32, :],
                )
    # ones columns (softmax denominator rows) written once
    for st in range(NS):
        nc.vector.memset(vt3[st][:, :, CH:CH + 1], 1.0)

    # full q or k production for one pair into a [128, T] psum tile
    # (4 x 256-col DoubleRow chains; one start per 2KB zero region)
    def emit_prod(j, which, pq):
        w_sb, bc_sb, dst_base = (
            (wq_sb, bqc_sb, q_f8[j][:, 0:]) if which == "q"
            else (wk_sb, bkc_sb, k_f8[j][:, S:])
        )
        wj = w_pair_view(w_sb, j)
        for c in range(4):
            for kp in range(NK):
                nc.tensor.matmul(
                    pq[:, 256 * c: 256 * (c + 1)],
                    lhsT=wj[:, kp],
                    rhs=xn_v[kp][:, :, 256 * c: 256 * (c + 1)],
                    start=(kp == 0 and c % 2 == 0), stop=(kp == NK - 1),
                    perf_mode=DR, skip_group_check=True,
                )
        nc.scalar.activation(
            dst_base[:, 0: T], pq[:, 0:T], ACT.Identity,
            bias=bc_sb[:, j: j + 1], scale=1.0,
        )
        emit_shuffle(j, which)

    # v^T production for one self t-tile into a [128, C]-wide psum view
    def emit_vt_tt(tt, pvt):
        for cs in range(NK):
            for kp in range(NK):
                nc.tensor.matmul(
                    pvt[:, 256 * cs: 256 * (cs + 1)],
                    lhsT=xn_v[kp][:, :, 128 * tt: 128 * (tt + 1)],
                    rhs=wv_v[:, kp, :, 256 * cs: 256 * (cs + 1)],
                    start=(kp == 0 and cs != 1), stop=(kp == NK - 1),
                    perf_mode=DR, skip_group_check=True,
                )
        nc.vector.tensor_scalar(
            vt3[2 + tt][:, :, 0:CH],
            pvt[:, 0:C].rearrange("p (h c) -> p h c", c=CH),
            APS / WSV, None, op0=AOP.mult,
        )

    # ============ phase A: enc matmuls (PE) + groupnorm (DVE/ACT) ============
    pA = tc.tile_pool(name="psumA", bufs=2, space="PSUM")
    pGN = tc.tile_pool(name="psumGN", bufs=1, space="PSUM")
    with pA as pa_pool, pGN as pgn_pool:
        def emit_enc():
            # enc keys -> k_sb[j][0:S]
            for j in range(NPAIR):
                pek = pa_pool.tile([128, S], F32, tag="pek", name=f"pek_{j}")
                wj = w_pair_view(wek_sb, j)
                for kp in range(NK):
                    nc.tensor.matmul(
                        pek[:], lhsT=wj[:, kp], rhs=enc_v[:, kp],
                        start=(kp == 0), stop=(kp == NK - 1), perf_mode=DR,
                    )
                nc.vector.tensor_scalar(
                    k_f8[j][:, 0:S], pek[:], 1.0,
                    bekc_sb[:, j: j + 1], op0=AOP.mult, op1=AOP.add,
                )

            # enc values transposed -> vt st 0..1 (+ ev bias via ones row)
            for st in range(2):
                pvt = pa_pool.tile([128, C], F32, tag="pvt", bufs=1,
                                   name=f"pvt_{st}")
                for cs in range(NK):
                    for kp in range(NK):
                        nc.tensor.matmul(
                            pvt[:, 256 * cs: 256 * (cs + 1)],
                            lhsT=enc_v[:, kp, :, 128 * st: 128 * (st + 1)],
                            rhs=wev_v[:, kp, :, 256 * cs: 256 * (cs + 1)],
                            start=(kp == 0 and cs != 1), stop=False,
                            perf_mode=DR, skip_group_check=True,
                        )
                    nc.tensor.matmul(
                        pvt[:, 256 * cs: 256 * (cs + 1)], lhsT=ones_r[0:1, :],
                        rhs=bev_sb[0:1, 256 * cs: 256 * (cs + 1)],
                        start=False, stop=True, skip_group_check=True,
                    )
                nc.vector.tensor_scalar(
                    vt3[st][:, :, 0:CH],
                    pvt[:].rearrange("p (h c) -> p h c", c=CH),
                    APS / WSV, None, op0=AOP.mult,
                )

        # ---- groupnorm stats (DVE/ACT; first PE work is the stats matmul) ----
        for ct in range(NP):
            xct = x_ct[ct][:]
            nc.vector.tensor_reduce(
                s12_sb[:, 2 * ct: 2 * ct + 1], xct, axis=mybir.AxisListType.X,
                op=AOP.add,
            )
            sq = sqpool.tile([128, T], F32, tag="sq", name=f"sq_{ct}")
            nc.scalar.activation(
                sq[:], xct, ACT.Square,
                accum_out=s12_sb[:, 2 * ct + 1: 2 * ct + 2],
            )
        pst = pgn_pool.tile([32, 2], F32, tag="pst")
        for ct in range(NP):
            nc.tensor.matmul(
                pst[:], lhsT=ind_sb[:, NG * ct: NG * (ct + 1)],
                rhs=s12_sb[:, 2 * ct: 2 * ct + 2],
                start=(ct == 0), stop=(ct == NP - 1),
            )
        n_per_group = (C // NG) * T
        gm = spool.tile([32, 1], F32, tag="gm")
        gm2 = spool.tile([32, 1], F32, tag="gm2")
        var_t = spool.tile([32, 1], F32, tag="var")
        ab32 = spool.tile([32, 2], F32, tag="ab32")
        nc.vector.tensor_scalar_mul(gm[:], pst[:, 0:1], 1.0 / n_per_group)
        nc.vector.tensor_tensor(gm2[:], gm[:], gm[:], op=AOP.mult)
        nc.vector.scalar_tensor_tensor(
            var_t[:], in0=pst[:, 1:2], scalar=1.0 / n_per_group, in1=gm2[:],
            op0=AOP.mult, op1=AOP.subtract,
        )
        v_t = spool.tile([32, 1], F32, tag="veps")
        nc.vector.tensor_scalar_add(v_t[:], var_t[:], float(EPS))
        y0i = spool.tile([32, 1], mybir.dt.int32, tag="y0i")
        nc.vector.tensor_scalar(
            y0i[:], v_t[:].bitcast(mybir.dt.int32), 1, None,
            op0=AOP.arith_shift_right,
        )
        nc.vector.tensor_scalar(
            y0i[:], y0i[:], -1, 0x5F3759DF, op0=AOP.mult, op1=AOP.add,
        )
        y = y0i[:].bitcast(F32)
        h_t = spool.tile([32, 1], F32, tag="half_v")
        nc.vector.tensor_scalar_mul(h_t[:], v_t[:], 0.5)
        yy = spool.tile([32, 1], F32, tag="yy")
        r_t = spool.tile([32, 1], F32, tag="rt")
        for it in range(3):
            nc.vector.tensor_tensor(yy[:], y, y, op=AOP.mult)
            nc.vector.tensor_tensor(r_t[:], h_t[:], yy[:], op=AOP.mult)
            nc.vector.tensor_scalar(
                r_t[:], r_t[:], -1.0, 1.5, op0=AOP.mult, op1=AOP.add,
            )
            dst = ab32[:, 0:1] if it == 2 else y
            nc.vector.tensor_tensor(dst, y, r_t[:], op=AOP.mult)
        nc.vector.scalar_tensor_tensor(
            ab32[:, 1:2], in0=gm[:], scalar=-1.0, in1=ab32[:, 0:1],
            op0=AOP.mult, op1=AOP.mult,
        )
        pab = pgn_pool.tile([128, 2 * NP], F32, tag="pab")
        for ct in range(NP):
            nc.tensor.matmul(
                pab[:, 2 * ct: 2 * ct + 2],
                lhsT=indT_sb[:, 128 * ct: 128 * (ct + 1)], rhs=ab32[:],
                start=True, stop=True, skip_group_check=True,
            )
        pab3 = pab[:].rearrange("p (ct two) -> p ct two", two=2)
        ab3 = ab_sb[:].rearrange("p (ct two) -> p ct two", two=2)
        gn3 = gnw_sb[:].rearrange("p (ct one) -> p ct one", one=1)
        gb3 = gnb_sb[:].rearrange("p (ct one) -> p ct one", one=1)
        nc.vector.tensor_tensor(ab3[:, :, 0:1], pab3[:, :, 0:1], gn3, op=AOP.mult)
        nc.vector.tensor_tensor(ab3[:, :, 1:2], pab3[:, :, 1:2], gn3, op=AOP.mult)
        nc.vector.tensor_tensor(ab3[:, :, 1:2], ab3[:, :, 1:2], gb3, op=AOP.add)
        for ct in range(NP):
            dst = xn_v[ct // 2][:, ct % 2, :]
            if ct % 2 == 0:
                nc.vector.tensor_scalar(
                    dst, x_ct[ct][:],
                    ab_sb[:, 2 * ct: 2 * ct + 1], ab_sb[:, 2 * ct + 1: 2 * ct + 2],
                    op0=AOP.mult, op1=AOP.add,
                )
            else:
                nc.scalar.activation(
                    dst, x_ct[ct][:],
                    ACT.Identity, bias=ab_sb[:, 2 * ct + 1: 2 * ct + 2],
                    scale=ab_sb[:, 2 * ct: 2 * ct + 1],
                )

        # pair-0 q/k production inside phase A (1 slot, serialized)
        for which in ("q", "k"):
            pq = pa_pool.tile([128, T], F32, tag="pp0", bufs=1,
                              name=f"pp0_{which}")
            emit_prod(0, which, pq)

        emit_enc()
        emit_shuffle(0, "k")

    sqpool.release()
    early.release()

    # ==== pair loop: each pair processed in two t-halves (120 steps) ====
    # Only 2 PV accumulators are live at a time, so banks split as:
    # pS 4x[128,512] (scores, 2-step drain slack) + pPV 2x[65,512] +
    # pX 1x[128,1024] (prod/vt insertions, off the scores rotation) = 8.
    pS = tc.alloc_tile_pool(name="psumS", bufs=4, space="PSUM")
    pPV = tc.alloc_tile_pool(name="psumPV", bufs=2, space="PSUM")
    pX = tc.alloc_tile_pool(name="psumX", bufs=1, space="PSUM")

    halves = [(j, th) for j in range(NPAIR) for th in range(2)]
    pa_t = {}
    E_tiles = {}
    exp_i = 0

    def emit_scores_exp(j, th, st):
        nonlocal exp_i
        for h in range(2):
            ps = pS.tile([128, 512], F32, tag="ps", name=f"ps_{j}_{th}_{st}_{h}")
            nc.tensor.matmul(
                ps[:],
                lhsT=kd_v[j][:, h, :, 128 * st: 128 * (st + 1)],
                rhs=qd_v[j][:, h, :, 512 * th: 512 * (th + 1)],
                start=True, stop=True, perf_mode=DR,
            )
            E_t = epool.tile([128, 512], BF16, tag="E", name=f"E_{j}_{th}_{st}_{h}")
            eng = EXP_SCHED[exp_i % len(EXP_SCHED)]
            exp_i += 1
            if eng == "g" and st < 3:
                # gpsimd is busy with the zrep broadcast around half-pair
                # starts; don't let exp tiles queue behind it
                eng = "a" if h == 0 else "d"
            if eng == "a":
                nc.scalar.activation(E_t[:], ps[:], ACT.Exp,
                                     scale=1.0 / (WS * WS))
            else:
                nc.vector.tensor_scalar(
                    E_t[:].bitcast(I16), ps[:],
                    EXP_A / (WS * WS), EXP_B, op0=AOP.mult, op1=AOP.add,
                )
            E_tiles[(j, th, st, h)] = E_t

    def emit_pv(j, th, st):
        # vt (with ones column) stationary, E moving:
        # pa[h] [65, 512] accumulates (a_raw; Z in row 64) over st
        for h in range(2):
            E_t = E_tiles.pop((j, th, st, h))
            nc.tensor.matmul(
                pa_t[(j, th)][h][:],
                lhsT=vt3[st][:, 2 * j + h, :], rhs=E_t[:],
                start=(st == 0), stop=(st == NS - 1),
            )

    def emit_normalize(j, th, tail=False):
        # stage pa out of PSUM into one [65, 1024] tile (frees banks for the
        # next half-pair; Z lands contiguous in row 64), broadcast raw Z,
        # then a_dr = cp / zrep in fp8 (x APS prescale already in vt)
        cp = zpool.tile([65, 1024], F32, tag="cp", name=f"cp_{j}_{th}")
        pa_jt = pa_t.pop((j, th))
        nc.scalar.activation(cp[:, 0:512], pa_jt[0][:], ACT.Copy)
        nc.vector.tensor_copy(cp[:, 512:1024], pa_jt[1][:])
        zcol = zpool.tile([1, 1024], F32, tag="zc", name=f"zc_{j}_{th}")
        nc.vector.reciprocal(zcol[:], cp[64:65, :])
        zrep = zpool.tile([64, 1024], F32, tag="zr", name=f"zr_{j}_{th}")
        nc.gpsimd.partition_broadcast(zrep[:], zcol[:])
        for h in range(2):
            eng = nc.vector if (tail and h == 0) else nc.gpsimd
            eng.tensor_tensor(
                a_v[j // 2][64 * h: 64 * (h + 1), j % 2,
                            512 * th: 512 * (th + 1)],
                cp[0:64, 512 * h: 512 * (h + 1)],
                zrep[:, 512 * h: 512 * (h + 1)],
                op=AOP.mult,
            )

    for hn, (j, th) in enumerate(halves):
        for st in range(NS):
            if st == 0:
                pa_t[(j, th)] = [
                    pPV.tile([65, 512], F32, tag="pa", name=f"pa_{j}_{th}_{h}")
                    for h in range(2)]
            emit_scores_exp(j, th, st)
            # PV schedule per half-pair: PV(0),PV(1) at st=3, PV(2..7) at
            # st=4..9, PV(8),PV(9) at next half's st=0,1; normalize of the
            # previous half at st=2 (cp lands a step before pa slot reuse).
            if st == 0 and hn >= 1:
                emit_pv(*halves[hn - 1], 8)
            elif st == 1 and hn >= 1:
                emit_pv(*halves[hn - 1], 9)
            elif st == 2 and hn >= 1:
                emit_normalize(*halves[hn - 1])
            elif st == 3:
                emit_pv(j, th, 0)
                emit_pv(j, th, 1)
            elif st >= 4:
                emit_pv(j, th, st - 2)
            # pair-0 vt production and next-pair q/k production run through
            # their own PSUM slot, off the scores rotation
            if hn == 0 and st < 8:
                pvt = pX.tile([128, T], F32, tag="px", name=f"pvs_{st}")
                emit_vt_tt(st, pvt)
            if j < NPAIR - 1 and th == 0 and st in (4, 8):
                which = "q" if st == 4 else "k"
                pq = pX.tile([128, T], F32, tag="px", name=f"pq_{j + 1}_{which}")
                emit_prod(j + 1, which, pq)

    # ============ tail: last PVs + normalize overlap proj partials ============
    def emit_proj_mms(ph, ot, half, kps):
        for c in (2 * half, 2 * half + 1):
            for kp in kps:
                nc.tensor.matmul(
                    ph[:, 256 * (c - 2 * half): 256 * (c - 2 * half) + 256],
                    lhsT=wp_v[:, kp, :, 128 * ot: 128 * (ot + 1)],
                    rhs=a_v[kp][:, :, 256 * c: 256 * (c + 1)],
                    start=(kp == 0 and c == 2 * half), stop=(kp == NK - 1),
                    perf_mode=DR, skip_group_check=True,
                )

    o_full = {}

    def emit_residual(ph, ot, half):
        # residuals per half (pipelines on DVE); one whole-row DMA per ot
        if ot not in o_full:
            o_full[ot] = opool.tile([128, T], F32, tag="out", bufs=3,
                                    name=f"o_{ot}")
        o_t = o_full[ot]
        nc.vector.scalar_tensor_tensor(
            o_t[:, 512 * half: 512 * (half + 1)], in0=ph[:],
            scalar=1.0 / (WSV * APS),
            in1=x_ct[ot][:, 512 * half: 512 * (half + 1)],
            op0=AOP.mult, op1=AOP.add,
        )
        if half == 1:
            eng = nc.sync if ot % 2 == 0 else nc.scalar
            eng.dma_start(out_ap[128 * ot: 128 * (ot + 1), :], o_t[:])

    ph01 = []
    for ot in (0, 1):
        for half in range(2):
            ph = pS.tile([128, 512], F32, tag="ps", name=f"ph_{ot}_{half}")
            emit_proj_mms(ph, ot, half, range(NK - 1))
            ph01.append((ph, ot, half))
    emit_pv(NPAIR - 1, 1, 8)
    emit_pv(NPAIR - 1, 1, 9)
    emit_normalize(NPAIR - 1, 1, tail=True)
    for ph, ot, half in ph01:
        emit_proj_mms(ph, ot, half, [NK - 1])
        emit_residual(ph, ot, half)
    for ot in range(2, NP):
        for half in range(2):
            ph = pS.tile([128, 512], F32, tag="ps", name=f"ph_{ot}_{half}")
            emit_proj_mms(ph, ot, half, range(NK))
            emit_residual(ph, ot, half)

    pX.release()
    pPV.release()
    pS.release()


def _prep_host(inputs):
    """Host-side weight prep. Returns (shared, per_core)."""
    x = np.ascontiguousarray(inputs["x"], dtype=np.float32).reshape(B, C, T)
    enc = np.ascontiguousarray(inputs["encoder_out"], dtype=np.float32)
    qkv_w = np.asarray(inputs["qkv_w"], np.float32)
    qkv_b = np.asarray(inputs["qkv_b"], np.float32)
    enc_w = np.asarray(inputs["enc_w"], np.float32)
    enc_b = np.asarray(inputs["enc_b"], np.float32)
    proj_w = np.asarray(inputs["proj_w"], np.float32)
    proj_b = np.asarray(inputs["proj_b"], np.float32)
    gn_w = np.asarray(inputs["gn_w"], np.float32)
    gn_b = np.asarray(inputs["gn_b"], np.float32)

    qkv_r = qkv_w.reshape(NH, 3 * CH, C)
    q_w = (qkv_r[:, :CH] * SCALE).reshape(NH * CH, C)
    k_w = (qkv_r[:, CH:2 * CH] * SCALE).reshape(NH * CH, C)
    v_w = qkv_r[:, 2 * CH:].reshape(NH * CH, C)
    qb = qkv_b.reshape(NH, 3 * CH)
    q_b = (qb[:, :CH] * SCALE).reshape(-1)
    k_b = (qb[:, CH:2 * CH] * SCALE).reshape(-1)
    v_b = qb[:, 2 * CH:].reshape(-1)
    enc_r = enc_w.reshape(NH, 2 * CH, C)
    ek_w = (enc_r[:, :CH] * SCALE).reshape(NH * CH, C)
    ev_w = enc_r[:, CH:].reshape(NH * CH, C)
    eb = enc_b.reshape(NH, 2 * CH)
    ek_b = (eb[:, :CH] * SCALE).reshape(-1)
    ev_b = eb[:, CH:].reshape(-1)
    hb = proj_w @ v_b + proj_b
    assert not np.any(hb), "nonzero v/proj bias not supported by v2 kernel"

    def dr_lhsT(w, scale):
        # w [out 768, in 768] -> [p, j, kp, i, m] = w[j*128+m, (2*kp+i)*128+p]
        a = (w * scale).reshape(6, 128, NK, 2, 128)   # [j, m, kp, i, p]
        a = a.transpose(4, 0, 2, 3, 1)                # [p, j, kp, i, m]
        return np.ascontiguousarray(a.reshape(128, -1)).astype(NPF8)

    def dr_rhs(w, scale):
        # w [out 768, in 768] -> [p, kp, i, n] = w[n, (2kp+i)*128+p]
        a = (w * scale).reshape(768, NK, 2, 128)      # [n, kp, i, p]
        a = a.transpose(3, 1, 2, 0)                   # [p, kp, i, n]
        return np.ascontiguousarray(a.reshape(128, -1)).astype(NPF8)

    ind = np.zeros((C, NG), np.float32)
    ind[np.arange(C), np.arange(C) // (C // NG)] = 1.0

    def colmaj(v):
        return np.ascontiguousarray(v.reshape(6, 128).T).astype(np.float32)

    shared = {
        "wq": dr_lhsT(q_w, WS), "wk": dr_lhsT(k_w, WS),
        "wek": dr_lhsT(ek_w, WS),
        "wv": dr_rhs(v_w, WSV), "wev": dr_rhs(ev_w, WSV),
        "wp": dr_lhsT(proj_w, WSV),
        "bev": (ev_b * WSV).reshape(1, C).astype(NPBF),
        "bqc": colmaj(q_b * WS), "bkc": colmaj(k_b * WS),
        "bekc": colmaj(ek_b * WS),
        "gnw": colmaj(gn_w), "gnb": colmaj(gn_b),
        "ind": ind, "indT": np.ascontiguousarray(ind.T),
        "idn": np.eye(128, dtype=NPBF),
    }
    per_core = []
    for b in range(B):
        e = enc[b].reshape(NK, 2, 128, S).transpose(2, 0, 1, 3).reshape(128, -1)
        per_core.append({
            "x": np.ascontiguousarray(x[b]),
            "enc": np.ascontiguousarray(e).astype(NPF8),
        })
    return shared, per_core


def _declare(nc):
    def di(name, shape, dt):
        return nc.dram_tensor(name, shape, dt, kind="ExternalInput").ap()

    ins = {
        "x": di("x", [C, T], F32),
        "enc": di("enc", [128, NK * 2 * S], FP8),
        "wq": di("wq", [128, NPAIR * NK * 2 * 128], FP8),
        "wk": di("wk", [128, NPAIR * NK * 2 * 128], FP8),
        "wek": di("wek", [128, NPAIR * NK * 2 * 128], FP8),
        "wv": di("wv", [128, NK * 2 * C], FP8),
        "wev": di("wev", [128, NK * 2 * C], FP8),
        "wp": di("wp", [128, NK * 2 * C], FP8),
        "bev": di("bev", [1, C], BF16),
        "bqc": di("bqc", [128, NPAIR], F32),
        "bkc": di("bkc", [128, NPAIR], F32),
        "bekc": di("bekc", [128, NPAIR], F32),
        "gnw": di("gnw", [128, NP], F32), "gnb": di("gnb", [128, NP], F32),
        "ind": di("ind", [C, NG], F32), "indT": di("indT", [NG, C], F32),
        "idn": di("idn", [128, 128], BF16),
    }
    out = nc.dram_tensor("out", [C, T], F32, kind="ExternalOutput").ap()
    return ins, out


def build_nc():
    nc = bacc.Bacc("TRN2", target_bir_lowering=False, debug=False)
    ins, out = _declare(nc)
    with tile.TileContext(nc) as tc:
        with ExitStack() as stack:
            tc._ctx = stack
            _emit(tc, ins, out)
    nc.compile()
    return nc


_NC_CACHE = {}


def run(inputs, trace=False):
    shared, per_core = _prep_host(inputs)
    if "nc" not in _NC_CACHE:
        _NC_CACHE["nc"] = build_nc()
    nc = _NC_CACHE["nc"]
    in_maps = [dict(shared, **pc) for pc in per_core]
    last_err = None
    for attempt in range(3):
        try:
            res = run_bass_kernel_spmd(nc, in_maps, list(range(B)), trace=trace)
            break
        except Exception as e:
            last_err = e
            if attempt == 2:
                raise
            import time
            time.sleep(15)
    outs = np.stack([r["out"] for r in res.results])  # [B, C, T]
    return outs.reshape(B, C, HH, WW).astype(np.float32), res


def kernel(**inputs):
    out, _ = run(inputs, trace=False)
    return out
